# revision 1
# baseline (speedup 1.0000x reference)
"""Soft-KNN Bass/Tile kernel for Trainium2 (8 NeuronCores) — v2.

Strategy
--------
- Host prep does ALL data conditioning: per-core shard sorted by label,
  transposed [dim, col] layouts, bf16 pair-split (or f32r rounding),
  y-norm ladder rows, query norms, class boundaries. Device setup = DMA only.
- Per core, per query-tile (128 queries x 16 tiles): accumulate
  z = X.y - yn into psum per 512-col chunk (ladder matmul + product
  passes), scalar-copy psum -> full fp32 z row [128, 6272] in SBUF.
- Selection: ONE vector.max (top-8 of whole row) + ONE max_index.
  Top-8 per core is enough: P(>8 of global top-16 in one shard) ~ 0.
  Labels from sorted-shard class boundaries (8x is_le+accum).
- AllGather of [2048, 16] f32 (8 z values + 8 labels per query per core),
  split into two collectives (qtiles 0-7 / 8-15) to overlap the tail.
  Core c owns qtiles {c, c+8}.
- Global: merge 64 candidates -> exact top-16 (max8/match_replace x2,
  cumsum-rank + gpsimd local_scatter compaction of value/label planes),
  d = sqrt(xn - z), softmax, scatter-add votes into 100 classes
  (vector for one owned qtile, gpsimd for the other - parallel tail).

Precision schemes (env KNN_SCHEME):
- exact13: x,y bf16 pair-split; 12 product passes; error ~3e-4 in z.
- f32r5:   x,y f32r (12-bit mantissa); 4 product passes; z noise ~8e-3.
"""

import ml_dtypes
import numpy as np

import concourse.bass as bass
import concourse.bacc as bacc
import concourse.mybir as mybir
import concourse.tile as tile
from concourse import bass_utils

F32 = mybir.dt.float32
F32R = mybir.dt.float32r
BF16 = mybir.dt.bfloat16
U16 = mybir.dt.uint16
I16 = mybir.dt.int16
AL = mybir.AluOpType
AF = mybir.ActivationFunctionType

NCORES = 8
B = 2048
D = 512
NTRAIN = 50000
COLS = 6400                    # 12*512 + 256; padded equal-boundary shard
CHUNKS = [512] * 12 + [256]
NCHUNK = len(CHUNKS)
QTILES = 16
NCLASS = 100
K = 16                         # global top-k
LK = 16                        # local candidates per core (8 per half-row)
NG = NCORES * LK               # 128 gathered candidates
HSPLIT = 6                     # chunks 0-5 -> half 0; 6-12 -> half 1
H0 = 512 * HSPLIT              # 3072
H1 = 6400 - H0                 # 3328
NEG = -3.0e38

SCHEME = "f32r5"
STAGE = 3


def _coff(c):
    return sum(CHUNKS[:c])


def build(scheme):
    nparts = 2 if scheme == "exact13" else 1     # x/y split parts
    ydt = BF16 if scheme == "exact13" else F32R
    nc = bacc.Bacc("TRN2", target_bir_lowering=False, num_devices=NCORES)

    # ---- dram inputs (host-prepped) ----
    # x side: [part, k, 128, 2048]; y side per chunk handled as separate tensors
    x_in = [[nc.dram_tensor(f"x{p}_{k}", [128, B], ydt, kind="ExternalInput")
             for k in range(4)] for p in range(nparts)]
    y_in = [[nc.dram_tensor(f"y{p}_{k}", [128, COLS], ydt, kind="ExternalInput")
             for k in range(4)] for p in range(nparts)]
    yn3_in = nc.dram_tensor("yn3", [4, COLS], BF16, kind="ExternalInput")
    xn_in = nc.dram_tensor("xn", [128, QTILES], F32, kind="ExternalInput")
    bnd_in = nc.dram_tensor("bnd", [128, NCLASS + 1], F32,
                            kind="ExternalInput")
    out_d = nc.dram_tensor("out", [3 * 128, NCLASS], F32, kind="ExternalOutput")

    ag_in = nc.dram_tensor("ag_in", [B, 2 * LK], F32)
    # 3 collectives: A=qt0-7 (fires mid-run), B=qt8-14, C=qt15 (tiny tail).
    # B end-padded 128 rows for pid 7's in-range junk read; C front-padded
    # 896 rows so the affine read offset c2*128 + pid*128 is always valid.
    ag_A = nc.dram_tensor("ag_A", [NCORES * 1024, 2 * LK], F32,
                          addr_space="Shared")
    ag_B = nc.dram_tensor("ag_B", [NCORES * 896 + 128, 2 * LK], F32,
                          addr_space="Shared")
    ag_C = nc.dram_tensor("ag_C", [NCORES * 128 + 896, 2 * LK], F32,
                          addr_space="Shared")

    with tile.TileContext(nc) as tc:
        with tc.tile_pool(name="res", bufs=1) as res, \
             tc.tile_pool(name="zps", bufs=7, space="PSUM") as zps, \
             tc.tile_pool(name="zrowp", bufs=1) as zrowp, \
             tc.tile_pool(name="small", bufs=3) as small:

            # ---- resident tiles (DMA-filled) ----
            xt = [[res.tile([128, B], ydt, name=f"xt{p}_{k}")
                   for k in range(4)] for p in range(nparts)]
            # y as per-chunk tiles for fine-grained DMA/compute overlap
            yt = [[[res.tile([128, CHUNKS[c]], ydt, name=f"yt{p}_{k}_{c}")
                    for c in range(NCHUNK)] for k in range(4)]
                  for p in range(nparts)]
            yn3 = res.tile([4, COLS], BF16)
            xn_all = res.tile([128, QTILES], F32)
            bnd_f = res.tile([128, NCLASS + 1], F32)
            ones4 = res.tile([4, 128], BF16)

            nc.vector.memset(ones4[:], 0.0)
            nc.vector.memset(ones4[0:3, :], -1.0)
            nc.sync.dma_start(yn3[:], yn3_in[:])
            nc.sync.dma_start(xn_all[:], xn_in[:])
            nc.sync.dma_start(bnd_f[:], bnd_in[:])
            # y chunks 0-1 first (qtile 0 compute), then x, then rest of y
            def _load_y(c):
                co = _coff(c)
                for p in range(nparts):
                    for k in range(4):
                        nc.sync.dma_start(yt[p][k][c][:],
                                          y_in[p][k][:, co:co + CHUNKS[c]])
            _load_y(0)
            _load_y(1)
            for p in range(nparts):
                for k in range(4):
                    nc.sync.dma_start(xt[p][k][:], x_in[p][k][:])
            for c in range(2, NCHUNK):
                _load_y(c)

            # product pass list: (x part, y part) pairs
            if scheme == "exact13":
                passes = [(0, 0), (0, 1), (1, 0)]
            else:
                passes = [(0, 0)]

            # ---- local phase (emission interleaved with collectives
            # and global phases so their engine-queue slots overlap) ----
            pid_sp = nc.sync.partition_id()
            agA3 = ag_A[:].rearrange("(c r) w -> c r w", c=NCORES)

            def do_qtile(qt):
                qs = qt * 128
                zh = [zrowp.tile([128, H0], F32, name=f"zh0_{qt}", tag="zh0"),
                      zrowp.tile([128, H1], F32, name=f"zh1_{qt}", tag="zh1")]
                for c in range(NCHUNK):
                    cw = CHUNKS[c]
                    co = _coff(c)
                    ps = zps.tile([128, 512], F32)
                    nc.tensor.matmul(ps[:, :cw], ones4[:], yn3[:, co:co + cw],
                                     start=True, stop=False)
                    for i, (px, py) in enumerate(passes):
                        for k in range(4):
                            last = (i == len(passes) - 1) and (k == 3)
                            nc.tensor.matmul(ps[:, :cw],
                                             xt[px][k][:, qs:qs + 128],
                                             yt[py][k][c][:, :cw],
                                             start=False, stop=last)
                    if c < HSPLIT:
                        nc.scalar.copy(zh[0][:, co:co + cw], ps[:, :cw])
                    else:
                        nc.scalar.copy(zh[1][:, co - H0:co - H0 + cw],
                                       ps[:, :cw])

                cv = small.tile([128, LK], F32, name=f"cv{qt}", tag="cv")
                ci = small.tile([128, LK], U16, name=f"ci{qt}", tag="ci")
                gf = small.tile([128, LK], F32, name=f"gf{qt}", tag="gf")
                for h in range(2):
                    s = slice(h * 8, h * 8 + 8)
                    nc.vector.max(cv[:, s], zh[h][:])
                    nc.vector.max_index(ci[:, s], cv[:, s], zh[h][:])
                nc.vector.tensor_copy(gf[:], ci[:])
                nc.vector.tensor_scalar(out=gf[:, 8:16], in0=gf[:, 8:16],
                                        scalar1=float(H0), scalar2=None,
                                        op0=AL.add)
                nc.sync.dma_start(ag_in[qs:qs + 128, 0:LK], cv[:])
                nc.sync.dma_start(ag_in[qs:qs + 128, LK:2 * LK], gf[:])

            def do_collective(t, lo, hi, pad):
                nc.gpsimd.collective_compute(
                    "AllGather", AL.bypass,
                    replica_groups=[list(range(NCORES))],
                    ins=[ag_in[lo:hi, :].opt()],
                    outs=[t[pad:pad + NCORES * (hi - lo), :].opt()])

            def do_phase(l):
                # l=0: qt=pid from A; l=1: qt=pid+8 from B (junk for pid 7);
                # l=2: qt=15 from C (real for pid 7 only).
                gvl = small.tile([128, NCORES, 2 * LK], F32,
                                 name=f"gvl{l}", tag="gvl")
                if l == 0:
                    nc.sync.dma_start(
                        gvl[:],
                        agA3[:, bass.ds(pid_sp * 128, 128), :]
                        .rearrange("c p w -> p c w"))
                else:
                    srct, stride = (ag_B, 896) if l == 1 else (ag_C, 128)
                    for c2 in range(NCORES):
                        nc.sync.dma_start(
                            gvl[:, c2, :],
                            srct[bass.ds(c2 * stride + pid_sp * 128, 128),
                                 :])
                gv = gvl[:, :, 0:LK]
                gl = gvl[:, :, LK:2 * LK]

                # exact top-16 of the 128 candidates + payload compaction
                t8a = small.tile([128, 8], F32, name=f"t8a{l}", tag="t8a")
                t8b = small.tile([128, 8], F32, name=f"t8b{l}", tag="t8b")
                m1 = small.tile([128, NG], F32, name=f"m1{l}", tag="m1")
                m2 = small.tile([128, NG], F32, name=f"m2{l}", tag="m2")
                nc.vector.max(t8a[:], gv)
                nc.vector.match_replace(m1[:], t8a[:], gv, NEG)
                nc.vector.max(t8b[:], m1[:])
                nc.vector.match_replace(m2[:], t8b[:], m1[:], NEG)
                mask = small.tile([128, NG], F32, name=f"mk{l}", tag="mk")
                nc.vector.tensor_scalar(out=mask[:], in0=m2[:],
                                        scalar1=-2e38, scalar2=None,
                                        op0=AL.is_le)
                csA = small.tile([128, NG], F32, name=f"csA{l}", tag="csA")
                csB = small.tile([128, NG], F32, name=f"csB{l}", tag="csB")
                nc.vector.tensor_copy(csA[:], mask[:])
                srcc, dst = csA, csB
                sh = 1
                while sh < NG:
                    nc.vector.tensor_copy(dst[:, 0:sh], srcc[:, 0:sh])
                    nc.vector.tensor_tensor(out=dst[:, sh:NG],
                                            in0=srcc[:, sh:NG],
                                            in1=srcc[:, 0:NG - sh],
                                            op=AL.add)
                    srcc, dst = dst, srcc
                    sh *= 2
                rk = small.tile([128, NG], F32, name=f"rk{l}", tag="rk")
                nc.vector.tensor_tensor(out=rk[:], in0=srcc[:],
                                        in1=mask[:], op=AL.mult)
                nc.vector.tensor_scalar(out=rk[:], in0=rk[:], scalar1=-1.0,
                                        scalar2=None, op0=AL.add)
                rk16 = small.tile([128, NG], I16, name=f"rk16{l}",
                                  tag="rk16")
                nc.vector.tensor_copy(rk16[:], rk[:])

                vlo = small.tile([128, NG], U16, name=f"vlo{l}", tag="vlo")
                vhi = small.tile([128, NG], U16, name=f"vhi{l}", tag="vhi")
                gvu = gv.bitcast(U16).rearrange("p a (b two) -> p a b two",
                                                two=2)
                nc.vector.tensor_copy(vlo[:], gvu[:, :, :, 0:1])
                nc.vector.tensor_copy(vhi[:], gvu[:, :, :, 1:2])
                glu = small.tile([128, NG], U16, name=f"glu{l}", tag="glu")
                nc.vector.tensor_copy(glu[:], gl)
                slo = small.tile([128, K], U16, name=f"slo{l}", tag="slo")
                shi = small.tile([128, K], U16, name=f"shi{l}", tag="shi")
                sla = small.tile([128, K], U16, name=f"sla{l}", tag="sla")
                for plane, out16 in ((vlo, slo), (vhi, shi), (glu, sla)):
                    nc.gpsimd.local_scatter(
                        out16[:].bitcast(I16), plane[:].bitcast(I16),
                        rk16[:], channels=128, num_elems=K, num_idxs=NG)
                v16 = small.tile([128, K], F32, name=f"v16{l}", tag="v16")
                v16u = v16[:].bitcast(U16).rearrange(
                    "p (a two) -> p a two", two=2)
                nc.vector.tensor_copy(v16u[:, :, 0:1], slo[:])
                nc.vector.tensor_copy(v16u[:, :, 1:2], shi[:])
                gidx = small.tile([128, K], F32, name=f"gx{l}", tag="gx")
                nc.vector.tensor_copy(gidx[:], sla[:])

                xn_col = small.tile([128, 1], F32, name=f"xnc{l}", tag="xnc")
                nc.sync.dma_start(
                    xn_col[:],
                    xn_all[:, bass.ds(pid_sp + (8 if l else 0), 1)])
                dsq = small.tile([128, K], F32, name=f"dsq{l}", tag="dsq")
                nc.scalar.activation(dsq[:], v16[:], AF.Sqrt, scale=-1.0,
                                     bias=xn_col[:, 0:1])
                ew = small.tile([128, K], F32, name=f"ew{l}", tag="ew")
                zsum = small.tile([128, 1], F32, name=f"zs{l}", tag="zs")
                nc.scalar.activation(ew[:], dsq[:], AF.Exp, scale=-1.0,
                                     accum_out=zsum[:, 0:1])
                rz = small.tile([128, 1], F32, name=f"rz{l}", tag="rz")
                nc.vector.reciprocal(rz[:], zsum[:])
                wt = small.tile([128, K], F32, name=f"wt{l}", tag="wt")
                nc.vector.tensor_scalar(out=wt[:], in0=ew[:],
                                        scalar1=rz[:, 0:1], scalar2=None,
                                        op0=AL.mult)
                # votes via cumulative difference: S_c = sum_r w_r *
                # [idx_r >= bnd_c]; vote_c = S_c - S_{c+1}.
                S = small.tile([128, NCLASS + 1], F32, name=f"S{l}", tag="S")
                tmp = small.tile([128, NCLASS + 1], F32, name=f"vt{l}",
                                 tag="vt")
                nc.vector.memset(S[:], 0.0)
                for r in range(K):
                    nc.vector.tensor_scalar(out=tmp[:], in0=bnd_f[:],
                                            scalar1=gidx[:, r:r + 1],
                                            scalar2=wt[:, r:r + 1],
                                            op0=AL.is_le, op1=AL.mult)
                    nc.vector.tensor_tensor(out=S[:], in0=S[:],
                                            in1=tmp[:], op=AL.add)
                vote = small.tile([128, NCLASS], F32, name=f"vote{l}",
                                  tag="vote")
                nc.vector.tensor_tensor(out=vote[:], in0=S[:, 0:NCLASS],
                                        in1=S[:, 1:NCLASS + 1],
                                        op=AL.subtract)
                nc.sync.dma_start(out_d[l * 128:(l + 1) * 128, :], vote[:])

            if STAGE >= 1:
                for qt in range(8):
                    do_qtile(qt)
                if STAGE >= 3:
                    do_collective(ag_A, 0, 1024, 0)
                do_qtile(8)
                if STAGE >= 3:
                    do_phase(0)
                for qt in range(9, 15):
                    do_qtile(qt)
                if STAGE >= 3:
                    do_collective(ag_B, 1024, 1920, 0)
                do_qtile(15)
                if STAGE >= 3:
                    do_collective(ag_C, 1920, 2048, 896)
                    do_phase(1)
                    do_phase(2)

    nc.finalize()
    return nc


def _bf16(a):
    """Round fp32 -> bf16 (round-to-nearest-even), keep fp32 container."""
    u = a.view(np.uint32)
    rounded = (u.astype(np.uint64) + 0x7FFF
               + ((u >> 16) & 1)) >> 16
    return (rounded.astype(np.uint32) << 16).view(np.float32)


def _f32r(a):
    i = a.view(np.uint32).astype(np.int64)
    i = (i + 0x400) & ~0x7FF
    return (i & 0xFFFFFFFF).astype(np.uint32).view(np.float32)


def _bf16_bits(a):
    """fp32 -> bf16 (ml_dtypes.bfloat16 array, round-to-nearest-even)."""
    u = a.view(np.uint32)
    rounded = (u.astype(np.uint64) + 0x7FFF + ((u >> 16) & 1)) >> 16
    return rounded.astype(np.uint16).view(ml_dtypes.bfloat16)


def _host_prep(x, tf, tl, scheme):
    """Produce per-core input dicts."""
    x = np.ascontiguousarray(np.asarray(x, np.float32))
    tf = np.ascontiguousarray(np.asarray(tf, np.float32))
    tl = np.asarray(tl, np.int64)

    X = 2.0 * x                                    # fold z = X.y - yn
    xn = np.sum(x.astype(np.float64) * x, axis=1).astype(np.float32)
    xn_all = xn.reshape(QTILES, 128).T.copy()      # [128, 16]

    if scheme == "exact13":
        X1 = _bf16(X); X2 = _bf16(X - X1)
        xparts = [X1, X2]
    else:
        xparts = [_f32r(X)]

    # equal-boundary deal: global sort by label, class c dealt round-robin
    # to cores, padded so every core has identical class boundaries.
    perm = np.argsort(tl, kind="stable")
    tf_s = tf[perm]
    counts = np.bincount(tl, minlength=NCLASS)
    m = (counts + NCORES - 1) // NCORES            # per-core padded counts
    assert m.sum() <= COLS, m.sum()
    bnd = np.concatenate([[0], np.cumsum(m)[:-1]])
    bnd_ext = np.concatenate([bnd.astype(np.float32) - 0.5, [1.0e9]])
    bnd_b = np.broadcast_to(bnd_ext, (128, NCLASS + 1)).copy()
    gstart = np.concatenate([[0], np.cumsum(counts)[:-1]])

    core_feats = np.zeros((NCORES, COLS, D), np.float32)
    core_valid = np.zeros((NCORES, COLS), bool)
    for cls in range(NCLASS):
        rows = tf_s[gstart[cls]:gstart[cls] + counts[cls]]
        for c in range(NCORES):
            part = rows[c::NCORES]
            core_feats[c, bnd[cls]:bnd[cls] + len(part)] = part
            core_valid[c, bnd[cls]:bnd[cls] + len(part)] = True

    in_maps = []
    for c in range(NCORES):
        feats = core_feats[c]
        yn = np.sum(feats.astype(np.float64) * feats,
                    axis=1).astype(np.float32)
        ynp = np.where(core_valid[c], yn, np.float32(1.0e30))
        y1 = _bf16(ynp); r1 = ynp - y1
        y2 = _bf16(r1); r2 = r1 - y2
        y3 = _bf16(r2)
        yn3 = np.zeros((4, COLS), ml_dtypes.bfloat16)
        yn3[0] = _bf16_bits(y1)
        yn3[1] = _bf16_bits(y2)
        yn3[2] = _bf16_bits(y3)

        if scheme == "exact13":
            Y1 = _bf16(feats); Y2 = _bf16(feats - Y1)
            yparts = [Y1, Y2]
        else:
            yparts = [_f32r(feats)]

        mm = {
            "yn3": yn3,
            "xn": xn_all,
            "bnd": bnd_b,
        }
        for p, xp in enumerate(xparts):
            # [2048, 512] -> k blocks [128, 2048]
            for k in range(4):
                blk = np.ascontiguousarray(xp[:, k * 128:(k + 1) * 128].T)
                mm[f"x{p}_{k}"] = (_bf16_bits(blk) if scheme == "exact13"
                                   else blk)
        for p, yp in enumerate(yparts):
            for k in range(4):
                blk = np.ascontiguousarray(yp[:, k * 128:(k + 1) * 128].T)
                mm[f"y{p}_{k}"] = (_bf16_bits(blk) if scheme == "exact13"
                                   else blk)
        in_maps.append(mm)
    return in_maps


_NC_CACHE = {}
LAST_RESULTS = None


def kernel(x, train_features, train_labels, **run_kwargs):
    global LAST_RESULTS
    in_maps = _host_prep(x, train_features, train_labels, SCHEME)
    if SCHEME not in _NC_CACHE:
        _NC_CACHE[SCHEME] = build(SCHEME)
    res = bass_utils.run_bass_kernel_spmd(
        _NC_CACHE[SCHEME], in_maps, core_ids=list(range(NCORES)),
        **run_kwargs)
    LAST_RESULTS = res
    out = np.zeros((B, NCLASS), np.float32)
    for c in range(NCORES):
        o = res.results[c]["out"]
        out[c * 128:(c + 1) * 128] = o[0:128]
        blk = 1 if c < 7 else 2
        out[(8 + c) * 128:(9 + c) * 128] = o[blk * 128:(blk + 1) * 128]
    return out

